# revision 35
# baseline (speedup 1.0000x reference)
"""TRN2 Bass kernel for nn_EdgeMLP: masked pairwise cosine similarity.

out[i, j] = [cls1_i == cls2_j] * cos(f(e1_i), f(e2_j)),  f = 2-layer MLP.

Strategy (8 cores, data-parallel over edges1 rows):
  - Host: sort edges2 columns by class label (pure data movement), so the
    class-equality mask becomes contiguous column segments.  Each core gets
    a 1024-row shard of edges1 and the full sorted edges2.
  - Device: MLP both sides (fp32 matmuls), column norms via a ones-matmul
    (replicated across 32 partitions), normalize, gate the edges1-side
    features per class (rows of wrong class zeroed), then one K=32 matmul
    per (class segment x 128-row tile x 512-col chunk).  Masked entries are
    exact zeros (gated lhsT column is all-zero).
  - Host: concatenate row shards, scatter columns back to original order.

MODE selects main-matmul precision:
  "f32"   exact fp32 (4 cyc/row)
  "f32r"  tf32-like fast mode (1 cyc/row, ~1.5e-4 rel err)
  "split" bf16 hi/lo 3-term split packed into one K=96 matmul
          (1 cyc/row, ~1e-5 rel err)
"""

import sys

for _p in ("/opt/trn_rl_repo", "/opt/pypackages"):
    if _p not in sys.path:
        sys.path.append(_p)

from contextlib import ExitStack

import ml_dtypes
import numpy as np

import concourse.bass as bass
import concourse.tile as tile
from concourse import bacc, mybir
from concourse.bass_utils import run_bass_kernel_spmd

F32 = mybir.dt.float32
F32R = mybir.dt.float32r
BF16 = mybir.dt.bfloat16
AF = mybir.ActivationFunctionType
ALU = mybir.AluOpType

N1, N2 = 8192, 8192
NCORES = 8
MLOC = N1 // NCORES  # 1024
DH, DF, NCLS = 64, 32, 8
CH = 512  # free-dim chunk (psum bank)

MODE = "split"

_cache: dict = {}


def _build_program(counts: tuple, mode: str, reps: int = 1):
    """Build the per-core Bacc program. `counts` = class histogram of the
    (sorted) edges2 columns; segment boundaries are baked into the loop
    structure. `reps` repeats the whole body (timing use only)."""
    bounds = np.concatenate([[0], np.cumsum(counts)]).astype(int)

    nc = bacc.Bacc("TRN2", target_bir_lowering=False, debug=False)

    e2t_d = nc.dram_tensor("e2t", [3, N2], F32, kind="ExternalInput").ap()
    e1t_d = nc.dram_tensor("e1t", [3, MLOC], F32, kind="ExternalInput").ap()
    cls1_d = nc.dram_tensor("cls1", [DF, MLOC], BF16, kind="ExternalInput").ap()
    w1_d = nc.dram_tensor("w1", [3, DH], F32, kind="ExternalInput").ap()
    b1_d = nc.dram_tensor("b1", [DH, 1], F32, kind="ExternalInput").ap()
    w2_d = nc.dram_tensor("w2", [DH, DF], F32, kind="ExternalInput").ap()
    b2_d = nc.dram_tensor("b2", [DF, 1], F32, kind="ExternalInput").ap()
    ones_d = nc.dram_tensor("ones", [DF, DF], F32, kind="ExternalInput").ap()
    out_d = nc.dram_tensor("out", [MLOC, N2], F32, kind="ExternalOutput").ap()

    with tile.TileContext(nc) as tc:
        for _rep in range(reps):
            _emit_body(nc, tc, bounds, mode,
                       e2t_d, e1t_d, cls1_d, w1_d, b1_d, w2_d, b2_d, ones_d,
                       out_d)

    nc.compile()
    return nc


def _emit_body(nc, tc, bounds, mode, e2t_d, e1t_d, cls1_d, w1_d, b1_d, w2_d,
               b2_d, ones_d, out_d):
    with ExitStack() as ctx:
        consts = ctx.enter_context(tc.tile_pool(name="consts", bufs=1))
        w1 = consts.tile([3, DH], F32)
        b1 = consts.tile([DH, 1], F32)
        w2 = consts.tile([DH, DF], F32)
        b2 = consts.tile([DF, 1], F32)
        ones = consts.tile([DF, DF], F32)
        cls1 = consts.tile([DF, MLOC], BF16)
        nc.sync.dma_start(w1[:], w1_d)
        nc.sync.dma_start(b1[:], b1_d)
        nc.sync.dma_start(w2[:], w2_d)
        nc.sync.dma_start(b2[:], b2_d)
        nc.sync.dma_start(ones[:], ones_d)
        nc.sync.dma_start(cls1[:], cls1_d)

        # persistent main-loop lhsT operand (gated edges1-side features)
        persist = ctx.enter_context(tc.tile_pool(name="persist", bufs=1))
        if mode == "split":
            v1m = persist.tile([3 * DF, NCLS, MLOC], BF16)  # [h1;l1;h1] gated
        elif mode == "f32r":
            v1m = persist.tile([DF, NCLS, MLOC], F32R)
        else:
            v1m = persist.tile([DF, NCLS, MLOC], F32)

        # side-1 pools stay open for the whole body (emission is interleaved
        # into the chunk loop below to avoid serializing the static per-engine
        # schedule on side-1's long dependency chain)
        scr1 = ctx.enter_context(tc.tile_pool(name="scr1", bufs=1))
        s1g = ctx.enter_context(tc.tile_pool(name="s1g", bufs=3))

        def side1_gen():
            """Yield after each instruction; computes v1m from e1t."""
            e1t = scr1.tile([3, MLOC], F32, tag="s1A")
            nc.sync.dma_start(e1t[:], e1t_d)
            yield
            hps1 = ppsum2.tile([DH, 2, CH], F32, tag="pps")
            for c0 in range(0, MLOC, CH):
                nc.tensor.matmul(hps1[:, c0 // CH, :], w1[:],
                                 e1t[:, c0:c0 + CH], start=True, stop=True)
            yield
            h1 = scr1.tile([DH, MLOC], F32, tag="s1B")
            nc.scalar.activation(h1[:], hps1[:].rearrange("p a b -> p (a b)"),
                                 AF.Relu, bias=b1[:], scale=1.0)
            yield
            fps1 = ppsum2.tile([DF, 2, CH], F32, tag="pps")
            for c0 in range(0, MLOC, CH):
                nc.tensor.matmul(fps1[:, c0 // CH, :], w2[:],
                                 h1[:, c0:c0 + CH], start=True, stop=True)
            yield
            sq1 = scr1.tile([DF, 2, CH], F32, tag="s1SQ")
            nc.scalar.activation(sq1[:], fps1[:], AF.Square, bias=b2[:],
                                 scale=1.0)
            yield
            nps1 = ppsum2.tile([DF, 2, CH], F32, tag="pps")
            for j in range(2):
                nc.tensor.matmul(nps1[:, j, :], ones[:], sq1[:, j, :],
                                 start=True, stop=True)
            yield
            nsq = scr1.tile([DF, MLOC], F32, tag="s1N")
            nc.scalar.sqrt(nsq[:], nps1[:].rearrange("p a b -> p (a b)"))
            yield
            nc.vector.reciprocal(nsq[:], nsq[:])
            yield
            u1 = scr1.tile([DF, MLOC], F32, tag="s1U")
            nc.vector.scalar_tensor_tensor(
                u1[:], fps1[:].rearrange("p a b -> p (a b)"), b2[:], nsq[:],
                ALU.add, ALU.mult)
            yield
            if mode == "split":
                hb1 = scr1.tile([DF, MLOC], BF16, tag="s1D")
                nc.scalar.copy(hb1[:], u1[:])
                yield
                rsd1 = scr1.tile([DF, MLOC], F32, tag="s1R")
                nc.vector.tensor_tensor(rsd1[:], u1[:], hb1[:], ALU.subtract)
                yield
                lb1 = scr1.tile([DF, MLOC], BF16, tag="s1E")
                nc.vector.tensor_copy(lb1[:], rsd1[:])
                yield
                for c in range(NCLS):
                    ghc = s1g.tile([DF, MLOC], BF16, tag="s1GH")
                    nc.vector.scalar_tensor_tensor(
                        ghc[:], cls1[:], float(c), hb1[:],
                        ALU.is_equal, ALU.mult)
                    nc.sync.dma_start(v1m[0:DF, c], ghc[:])
                    nc.sync.dma_start(v1m[2 * DF:3 * DF, c], ghc[:])
                    yield
                    glc = s1g.tile([DF, MLOC], BF16, tag="s1GL")
                    nc.vector.scalar_tensor_tensor(
                        glc[:], cls1[:], float(c), lb1[:],
                        ALU.is_equal, ALU.mult)
                    nc.sync.dma_start(v1m[DF:2 * DF, c], glc[:])
                    yield
            else:
                if mode == "f32":
                    v1g = v1m
                else:
                    v1g = scr1.tile([DF, NCLS, MLOC], F32, tag="s1G")
                for c in range(NCLS):
                    nc.vector.scalar_tensor_tensor(
                        v1g[:, c, :], cls1[:], float(c), u1[:],
                        ALU.is_equal, ALU.mult)
                    yield
                if mode == "f32r":
                    nc.vector.tensor_copy(v1m[:], v1g[:])

        # ---- pipelined side-2 + main loop, one 1024-col chunk at a time ----
        # (prologue fp32 matmuls sub-chunk at 512 = fp32 moving-max; all
        # elementwise/copy/DMA ops run at 1024 free for half the instruction
        # overheads and 4KB-contiguous output rows)
        CHO = 2 * CH
        e2pool = ctx.enter_context(tc.tile_pool(name="e2p", bufs=1))
        e2t = e2pool.tile([3, N2], F32)
        nc.sync.dma_start(e2t[:], e2t_d)

        cpool = ctx.enter_context(tc.tile_pool(name="cscr", bufs=2))
        v2pool = ctx.enter_context(tc.tile_pool(name="v2p", bufs=6))
        ppsum2 = ctx.enter_context(tc.tile_pool(name="ppsum2", bufs=2, space="PSUM"))
        mpsum = ctx.enter_context(tc.tile_pool(name="mpsum", bufs=2, space="PSUM"))
        opool = ctx.enter_context(tc.tile_pool(name="osb", bufs=6))
        n_mt = MLOC // 128
        n_chunks = N2 // CHO

        s1 = side1_gen()
        s1_done = False

        def s1_steps(k):
            nonlocal s1_done
            for _ in range(k):
                if next(s1, "end") == "end":
                    s1_done = True
                    return

        def emit_pro_a(chi):
            """MLP + squared-norm matmuls for 1024-col chunk chi."""
            lo = chi * CHO
            hps = ppsum2.tile([DH, 2, CH], F32, tag="pps")
            for j in range(2):
                nc.tensor.matmul(hps[:, j, :], w1[:],
                                 e2t[:, lo + j * CH:lo + (j + 1) * CH],
                                 start=True, stop=True)
            h = cpool.tile([DH, 2, CH], F32, tag="h")
            nc.scalar.activation(h[:], hps[:], AF.Relu, bias=b1[:], scale=1.0)
            fps = ppsum2.tile([DF, 2, CH], F32, tag="pps")
            for j in range(2):
                nc.tensor.matmul(fps[:, j, :], w2[:], h[:, j, :],
                                 start=True, stop=True)
            # f^2 = Square(fps + b2) straight from PSUM (f itself is never
            # materialized; u below re-reads fps)
            sq = cpool.tile([DF, 2, CH], F32, tag="sq")
            nc.scalar.activation(sq[:], fps[:], AF.Square, bias=b2[:], scale=1.0)
            nps = ppsum2.tile([DF, 2, CH], F32, tag="pps")
            for j in range(2):
                nc.tensor.matmul(nps[:, j, :], ones[:], sq[:, j, :],
                                 start=True, stop=True)
            rn = cpool.tile([DF, CHO], F32, tag="rn")
            nc.scalar.sqrt(rn[:], nps[:].rearrange("p a b -> p (a b)"))
            nc.vector.reciprocal(rn[:], rn[:])
            return fps, rn

        def emit_pro_b(chi, frn):
            """normalize + (hi/lo split) -> v2 for chunk chi."""
            fps, rn = frn
            u = cpool.tile([DF, CHO], F32, tag="u")
            # u = (fps + b2) * rn  -- bias-add and normalize fused, from PSUM
            nc.vector.scalar_tensor_tensor(
                u[:], fps[:].rearrange("p a b -> p (a b)"), b2[:], rn[:],
                ALU.add, ALU.mult)
            if mode == "split":
                v2 = v2pool.tile([3 * DF, CHO], BF16, tag="v2")
                # hi part straight into section 0 (lane-aligned with u)
                nc.scalar.copy(v2[0:DF, :], u[:])
                # residual: mixed-dtype subtract reads the bf16 hi back
                rsd = cpool.tile([DF, CHO], F32, tag="rsd")
                nc.vector.tensor_tensor(rsd[:], u[:], v2[0:DF, :], ALU.subtract)
                lb = cpool.tile([DF, CHO], BF16, tag="lb")
                nc.vector.tensor_copy(lb[:], rsd[:])
                # duplicate hi into section 1, lo into section 2 (partition
                # moves need DMA)
                nc.sync.dma_start(v2[DF:2 * DF, :], v2[0:DF, :])
                nc.sync.dma_start(v2[2 * DF:3 * DF, :], lb[:])
            elif mode == "f32r":
                v2 = v2pool.tile([DF, CHO], F32R, tag="v2")
                nc.vector.tensor_copy(v2[:], u[:])
            else:
                v2 = v2pool.tile([DF, CHO], F32, tag="v2")
                nc.vector.tensor_copy(v2[:], u[:])
            return v2

        def emit_main(chi, v2):
            lo, hi = chi * CHO, (chi + 1) * CHO
            pieces = []
            for c in range(NCLS):
                a, b = max(lo, bounds[c]), min(hi, bounds[c + 1])
                if a < b:
                    pieces.append((c, a, b))
            for m in range(n_mt):
                ps = mpsum.tile([128, CHO], F32)
                for (c, a, b) in pieces:
                    # split on the absolute 512-col grid: each matmul must
                    # stay inside one psum bank (and under the ISA
                    # moving-elements limit)
                    a2 = a
                    while a2 < b:
                        b2 = min(b, (a2 - lo) // CH * CH + lo + CH)
                        nc.tensor.matmul(
                            ps[:, a2 - lo:b2 - lo],
                            v1m[:, c, m * 128:(m + 1) * 128],
                            v2[:, a2 - lo:b2 - lo],
                            start=True, stop=True)
                        a2 = b2
                ob = opool.tile([128, CHO], F32)
                if (chi + m) % 2 == 0:
                    nc.scalar.copy(ob[:], ps[:])
                else:
                    nc.vector.tensor_copy(ob[:], ps[:])
                nc.sync.dma_start(out_d[m * 128:(m + 1) * 128, lo:hi], ob[:])

        # interleaved emission: side-1 steps ride along the first chunks'
        # prologues; mains lag the prologue stream by LAG chunks so prologue
        # chain latency stays off the critical path.
        LAG = 2
        v2s = {}
        nxt = 0
        for chi in range(n_chunks):
            if not s1_done:
                s1_steps(10)
            v2s[chi] = emit_pro_b(chi, emit_pro_a(chi))
            if chi + 1 >= LAG and s1_done and nxt <= chi - LAG + 1:
                emit_main(nxt, v2s.pop(nxt))
                nxt += 1
        if not s1_done:
            s1_steps(1000)
        while nxt < n_chunks:
            emit_main(nxt, v2s.pop(nxt))
            nxt += 1


def kernel(**inputs) -> np.ndarray:
    edges1 = np.ascontiguousarray(np.asarray(inputs["edges1"], dtype=np.float32))
    edges2 = np.ascontiguousarray(np.asarray(inputs["edges2"], dtype=np.float32))
    W1 = np.asarray(inputs["W1"], dtype=np.float32)
    b1 = np.asarray(inputs["b1"], dtype=np.float32)
    W2 = np.asarray(inputs["W2"], dtype=np.float32)
    b2 = np.asarray(inputs["b2"], dtype=np.float32)

    cls2 = edges2[:, 3].astype(np.int64)
    order = np.argsort(cls2, kind="stable")
    counts = tuple(int(x) for x in np.bincount(cls2, minlength=NCLS))

    key = (counts, MODE)
    if key not in _cache:
        _cache[key] = _build_program(counts, MODE)
    nc = _cache[key]

    e2s = edges2[order]
    e2t = np.ascontiguousarray(e2s[:, :3].T)  # [3, N2]
    shared = {
        "e2t": e2t,
        "w1": W1,
        "b1": np.ascontiguousarray(b1[:, None]),
        "w2": W2,
        "b2": np.ascontiguousarray(b2[:, None]),
        "ones": np.ones((DF, DF), dtype=np.float32),
    }
    in_maps = []
    for k in range(NCORES):
        sl = slice(k * MLOC, (k + 1) * MLOC)
        e1t = np.ascontiguousarray(edges1[sl, :3].T)  # [3, MLOC]
        c1 = np.ascontiguousarray(
            np.broadcast_to(edges1[sl, 3][None, :], (DF, MLOC))
        ).astype(ml_dtypes.bfloat16)
        in_maps.append({**shared, "e1t": e1t, "cls1": c1})

    res = run_bass_kernel_spmd(nc, in_maps, core_ids=list(range(NCORES)))
    out_sorted = np.concatenate(
        [res.results[k]["out"] for k in range(NCORES)], axis=0)
    out = np.empty((N1, N2), dtype=np.float32)
    out[:, order] = out_sorted
    return out
